# revision 21
# baseline (speedup 1.0000x reference)
"""Bass/Trainium2 kernel for nn_ClassQueryAttention.

Math (per batch b, x flattened to [C=256, N=16384]):
  lT[n,k]  = x_sub.T @ (qe @ Wk).T / sqrt(D)      (transposed logits; qe@bk cancels)
  pT = exp(lT)  (no max-subtraction needed: logits ~ N(0,1))
  [y | s] accumulated flash-style in PSUM: y[k,c] = sum_n pT[n,k] xT[n,c],
          s[k] = sum_n pT[n,k]  (ones column appended to xT)
  xa[c]  = sum_k y[k,c] / s[k]
  gate   = (Wo@Wv) @ xa + K*(Wo@bv + bo)
  out[c,n] = x[c,n] * gate[c]

Sharding: data-parallel over batch B=8, one batch per NeuronCore, no collectives.
bf16 datapath (tolerance 2e-2): per-core HBM traffic 8 MiB in + 8 MiB out.
"""

import sys
from contextlib import ExitStack

import numpy as np

sys.path.insert(0, "/opt/trn_rl_repo")

import concourse.bass as bass  # noqa: E402
import concourse.tile as tile  # noqa: E402
from concourse import bacc, mybir  # noqa: E402
from concourse.bass_utils import run_bass_kernel_spmd  # noqa: E402

B, C, HW = 8, 256, 128 * 128
K, D = 21, 256
P = 128          # partition count / channel chunk
NB = 2048        # DMA big-tile pixels
NQ = 512         # quad pixels
NS = 128         # transpose subtile pixels
F32 = mybir.dt.float32
BF16 = mybir.dt.bfloat16
AF = mybir.ActivationFunctionType


def _body(ctx: ExitStack, tc: tile.TileContext, x, qk, m2, g0, ident, sel, selt,
          out, sfx=""):
    nc = tc.nc

    def pool(name, **kw):
        return ctx.enter_context(tc.tile_pool(name=name + sfx, **kw))

    consts = pool("consts", bufs=1)
    qk0 = consts.tile([P, K], BF16, tag="qk0")
    qk1 = consts.tile([P, K], BF16, tag="qk1")
    m2t0 = consts.tile([P, C], F32, tag="m2t0")
    m2t1 = consts.tile([P, C], F32, tag="m2t1")
    g0_sb = consts.tile([P, 2], F32, tag="g0")
    id_sb = consts.tile([P, P], BF16, tag="ident")
    sel_sb = consts.tile([P, K], F32, tag="sel")    # sel[32j+k, k] = 1
    selt_sb = consts.tile([K, P], F32, tag="selt")  # its transpose

    xbig = pool("xbig", bufs=1)
    xres = {}
    xchunk = {}
    n_big = HW // NB                 # 8
    for g in range(NB // NQ):        # big tile 0 arrives as quad-sized chunks
        xc0_t = xbig.tile([P, NQ], BF16, tag=f"xc0_{g}")
        xc1_t = xbig.tile([P, NQ], BF16, tag=f"xc1_{g}")
        xchunk[0, g] = xc0_t
        xchunk[1, g] = xc1_t
    for bt in range(1, n_big):
        xb0_t = xbig.tile([P, NB], BF16, tag=f"xb0_{bt}")
        xb1_t = xbig.tile([P, NB], BF16, tag=f"xb1_{bt}")
        xres[0, bt] = xb0_t
        xres[1, bt] = xb1_t

    # first quad's chunk + early-needed consts first so the pipeline head
    # starts ASAP; phase-B-only consts last.
    nc.sync.dma_start(xchunk[0, 0][:], x[0:P, 0:NQ])
    nc.sync.dma_start(xchunk[1, 0][:], x[P : 2 * P, 0:NQ])
    nc.sync.dma_start(qk0[:], qk[0:P, :])
    nc.sync.dma_start(qk1[:], qk[P : 2 * P, :])
    nc.sync.dma_start(id_sb[:], ident[:, :])
    for g in range(1, NB // NQ):
        nc.sync.dma_start(xchunk[0, g][:], x[0:P, g * NQ : (g + 1) * NQ])
        nc.sync.dma_start(xchunk[1, g][:], x[P : 2 * P, g * NQ : (g + 1) * NQ])
    for bt in range(1, n_big):
        nc.sync.dma_start(xres[0, bt][:], x[0:P, bt * NB : (bt + 1) * NB])
        nc.sync.dma_start(xres[1, bt][:], x[P : 2 * P, bt * NB : (bt + 1) * NB])
    nc.sync.dma_start(m2t0[:], m2[0:P, :])
    nc.sync.dma_start(m2t1[:], m2[P : 2 * P, :])
    nc.sync.dma_start(g0_sb[:], g0[:, :])
    nc.sync.dma_start(sel_sb[:], sel[:, :])
    nc.sync.dma_start(selt_sb[:], selt[:, :])
    ps_l = pool("ps_l", bufs=2, space="PSUM")      # lT [128,84] f32
    ps_xt = pool("ps_xt", bufs=4, space="PSUM")    # xT half [128,2,256] bf16
    ps_y = pool("ps_y", bufs=1, space="PSUM")      # [y|s] accum [128,257] f32
    sb_xt = pool("sb_xt", bufs=4)
    sb_pt = pool("sb_pt", bufs=3)

    # ---------------- Phase A ------------------------------------------------
    # x stays resident in SBUF (16 tiles x 4 KiB/partition) -> phase C needs no
    # second HBM read. Per 512-px quad: 8 transposed-logits matmuls (x subtile
    # stationary, qk moving, 21 rows each) -> one exp [128,84] writing pT
    # straight to SBUF -> 8 x-transposes -> 4 y matmuls (col-group packed,
    # rhs gets a ones column so row 256 of PSUM accumulates s). The y matmuls
    # are emitted one quad late so the PE never stalls on the copy chain.
    # full-bank rows (512 f32 = 2048 B) so each col-group's partition offset is
    # zero-region aligned; cols past 257 are unused padding
    y_ps = ps_y.tile([P, 2 * C], F32, tag="y")
    nc.vector.memset(y_ps[:, :], 0.0)   # zero padding rows; y groups overwrite
    n_quad = NB // NQ                # 4
    n_sub = NQ // NS                 # 4
    n_t = HW // NQ                   # 32
    pend = None                      # deferred y-matmul args for quad t-1

    def emit_y(pt_sb, xt_sb, t):
        for j in range(n_sub):
            nc.tensor.matmul(
                y_ps[32 * j : 32 * j + K, 0 : C + 1],
                pt_sb[:, j * K : (j + 1) * K],
                xt_sb[:, j, :],
                start=(t == 0),
                stop=(t == n_t - 1),
                skip_group_check=True,
                tile_position=(0, 32 * j),
            )

    for bt in range(n_big):
        for g in range(n_quad):
            t = bt * n_quad + g
            if bt == 0:
                xb0, xb1, base = xchunk[0, g], xchunk[1, g], 0
            else:
                xb0, xb1, base = xres[0, bt], xres[1, bt], g * NQ
            # transposed logits: lT[n, k] per 128-px subtile, both c-halves
            # accumulate. x subtile is the stationary operand.
            l_ps = ps_l.tile([P, n_sub * K], F32, tag="l")
            for j in range(n_sub):
                ss = slice(base + j * NS, base + (j + 1) * NS)
                osl = slice(j * K, (j + 1) * K)
                nc.tensor.matmul(l_ps[:, osl], xb0[:, ss], qk0[:],
                                 start=True, stop=False, skip_group_check=True)
                nc.tensor.matmul(l_ps[:, osl], xb1[:, ss], qk1[:],
                                 start=False, stop=True, skip_group_check=True)
            # exp -> pT directly in SBUF (bf16)
            pt_sb = sb_pt.tile([P, n_sub * K], BF16, tag="ptsb")
            nc.scalar.activation(pt_sb[:], l_ps[:], AF.Exp)

            # x transposes in two halves; copies all on DVE; ones column for s
            xt_sb = sb_xt.tile([P, n_sub, C + 1], BF16, tag="xtsb")
            nc.vector.memset(xt_sb[:, :, C : C + 1], 1.0)
            for h in range(2):
                xt_ps = ps_xt.tile([P, 2, C], BF16, tag="xt")
                for jj in range(2):
                    j = 2 * h + jj
                    ss = slice(base + j * NS, base + (j + 1) * NS)
                    nc.tensor.transpose(xt_ps[:, jj, 0:P], xb0[:, ss], id_sb[:, :])
                    nc.tensor.transpose(xt_ps[:, jj, P : 2 * P], xb1[:, ss], id_sb[:, :])
                nc.vector.tensor_copy(xt_sb[:, 2 * h : 2 * h + 2, 0:C], xt_ps[:])

            if pend is not None:
                emit_y(*pend)
            pend = (pt_sb, xt_sb, t)

    emit_y(*pend)

    # ---------------- Phase B: s -> r -> xa -> gate ---------------------------
    # y lives in 4 col-group blocks at partitions 32j..32j+20; each group holds
    # PARTIAL sums over its own subtile pixels, col 256 the partial s. Reduce
    # partial s across groups (sel.T @ s4), invert, replicate back to groups.
    s4_sb = consts.tile([P, 1], F32, tag="s4_sb")
    nc.vector.tensor_copy(s4_sb[:], y_ps[:, C : C + 1])
    yf_sb = consts.tile([P, C], F32, tag="yf_sb")
    nc.scalar.copy(yf_sb[:], y_ps[:, 0:C])
    sg_ps = ps_l.tile([K, 1], F32, tag="l")
    nc.tensor.matmul(sg_ps[:], sel_sb[:], s4_sb[:], start=True, stop=True)
    r_sb = consts.tile([K, 1], F32, tag="r_sb")
    nc.vector.reciprocal(r_sb[:], sg_ps[:])
    r4_ps = ps_l.tile([P, 1], F32, tag="l")
    nc.tensor.matmul(r4_ps[:], selt_sb[:], r_sb[:], start=True, stop=True)
    r4_sb = consts.tile([P, 1], F32, tag="r4_sb")
    nc.vector.tensor_copy(r4_sb[:], r4_ps[:])

    # xaT[m, h] = sum_p r4[p] yf[p, 128h+m]: two 1-col matmuls, no transposes
    xat_ps = ps_l.tile([P, 2], F32, tag="l")
    for h in range(2):
        nc.tensor.matmul(xat_ps[:, h : h + 1], yf_sb[:, h * P : (h + 1) * P],
                         r4_sb[:], start=True, stop=True, skip_group_check=True)
    xat_sb = consts.tile([P, 2], F32, tag="xat_sb")
    nc.vector.tensor_copy(xat_sb[:], xat_ps[:])

    gate_ps = ps_l.tile([P, 2], F32, tag="l")
    for cc in range(2):
        csl = slice(cc * P, (cc + 1) * P)
        nc.tensor.matmul(
            gate_ps[:, cc : cc + 1], m2t0[:, csl], xat_sb[:, 0:1],
            start=True, stop=False, skip_group_check=True,
        )
        nc.tensor.matmul(
            gate_ps[:, cc : cc + 1], m2t1[:, csl], xat_sb[:, 1:2],
            start=False, stop=True, skip_group_check=True,
        )
    gate_sb = consts.tile([P, 2], F32, tag="gate_sb")
    nc.vector.tensor_add(gate_sb[:], gate_ps[:], g0_sb[:])

    # ---------------- Phase C: out = x * gate (x already in SBUF) -------------
    for cc in range(2):
        csl = slice(cc * P, (cc + 1) * P)
        for g in range(NB // NQ):
            xc = xchunk[cc, g]
            nc.vector.tensor_scalar_mul(xc[:], xc[:], gate_sb[:, cc : cc + 1])
            nc.sync.dma_start(out[csl, g * NQ : (g + 1) * NQ], xc[:])
        for nt in range(1, HW // NB):
            xc = xres[cc, nt]
            nsl = slice(nt * NB, (nt + 1) * NB)
            nc.vector.tensor_scalar_mul(xc[:], xc[:], gate_sb[:, cc : cc + 1])
            nc.sync.dma_start(out[csl, nsl], xc[:])


def build_nc(repeats=1, body=None):
    body = body or _body
    nc = bacc.Bacc(
        "TRN2",
        target_bir_lowering=False,
        debug=False,
        enable_asserts=False,
        num_devices=B,
    )
    x = nc.dram_tensor("x", [C, HW], BF16, kind="ExternalInput").ap()
    qk = nc.dram_tensor("qkT", [C, K], BF16, kind="ExternalInput").ap()
    m2 = nc.dram_tensor("m2t", [C, C], F32, kind="ExternalInput").ap()
    g0 = nc.dram_tensor("g0", [P, 2], F32, kind="ExternalInput").ap()
    ident = nc.dram_tensor("ident", [P, P], BF16, kind="ExternalInput").ap()
    sel = nc.dram_tensor("sel", [P, K], F32, kind="ExternalInput").ap()
    selt = nc.dram_tensor("selt", [K, P], F32, kind="ExternalInput").ap()
    out = nc.dram_tensor("out", [C, HW], BF16, kind="ExternalOutput").ap()

    with tile.TileContext(nc) as tc:
        for r in range(repeats):
            with ExitStack() as ctx:
                body(ctx, tc, x, qk, m2, g0, ident, sel, selt, out, sfx=f"_{r}")
    nc.compile()
    return nc


_NC = None


def _get_nc():
    global _NC
    if _NC is None:
        _NC = build_nc()
    return _NC


def make_in_maps(x, query_embed, Wk, bk, Wv, bv, Wo, bo):
    import ml_dtypes

    x = np.asarray(x, dtype=np.float32)
    qe = np.asarray(query_embed, dtype=np.float64)
    Wk64 = np.asarray(Wk, dtype=np.float64)
    Wv64 = np.asarray(Wv, dtype=np.float64)
    Wo64 = np.asarray(Wo, dtype=np.float64)
    bv64 = np.asarray(bv, dtype=np.float64)
    bo64 = np.asarray(bo, dtype=np.float64)

    qkT = ((qe @ Wk64) / np.sqrt(float(D))).T.astype(ml_dtypes.bfloat16).copy()
    m2t = (Wo64 @ Wv64).T.astype(np.float32).copy()
    g0 = (float(K) * (Wo64 @ bv64 + bo64)).astype(np.float32)
    g0c = np.ascontiguousarray(g0.reshape(2, P).T)
    ident = np.eye(P, dtype=ml_dtypes.bfloat16)
    sel = np.zeros((P, K), dtype=np.float32)
    for g in range(4):
        for k in range(K):
            sel[32 * g + k, k] = 1.0
    selt = np.ascontiguousarray(sel.T)

    return [
        {
            "x": np.ascontiguousarray(x[b].reshape(C, HW).astype(ml_dtypes.bfloat16)),
            "qkT": qkT,
            "m2t": m2t,
            "g0": g0c,
            "ident": ident,
            "sel": sel,
            "selt": selt,
        }
        for b in range(B)
    ]


def kernel(x, query_embed, Wk, bk, Wv, bv, Wo, bo, _trace=False, **kw):
    in_maps = make_in_maps(x, query_embed, Wk, bk, Wv, bv, Wo, bo)
    nc = _get_nc()
    res = run_bass_kernel_spmd(nc, in_maps, core_ids=list(range(B)), trace=_trace, **kw)
    out = np.stack(
        [np.asarray(res.results[b]["out"]).astype(np.float32).reshape(C, 128, 128)
         for b in range(B)]
    )
    if _trace:
        kernel.last_results = res
    return out


# revision 22
# speedup vs baseline: 1.0601x; 1.0601x over previous
"""Bass/Trainium2 kernel for nn_ClassQueryAttention.

Math (per batch b, x flattened to [C=256, N=16384]):
  lT[n,k]  = x_sub.T @ (qe @ Wk).T / sqrt(D)      (transposed logits; qe@bk cancels)
  pT = exp(lT)  (no max-subtraction needed: logits ~ N(0,1))
  [y | s] accumulated flash-style in PSUM: y[k,c] = sum_n pT[n,k] xT[n,c],
          s[k] = sum_n pT[n,k]  (ones column appended to xT)
  xa[c]  = sum_k y[k,c] / s[k]
  gate   = (Wo@Wv) @ xa + K*(Wo@bv + bo)
  out[c,n] = x[c,n] * gate[c]

Sharding: data-parallel over batch B=8, one batch per NeuronCore, no collectives.
bf16 datapath (tolerance 2e-2): per-core HBM traffic 8 MiB in + 8 MiB out.
"""

import sys
from contextlib import ExitStack

import numpy as np

sys.path.insert(0, "/opt/trn_rl_repo")

import concourse.bass as bass  # noqa: E402
import concourse.tile as tile  # noqa: E402
from concourse import bacc, mybir  # noqa: E402
from concourse.bass_utils import run_bass_kernel_spmd  # noqa: E402

B, C, HW = 8, 256, 128 * 128
K, D = 21, 256
P = 128          # partition count / channel chunk
NB = 2048        # DMA big-tile pixels
NQ = 512         # quad pixels
NS = 128         # transpose subtile pixels
F32 = mybir.dt.float32
BF16 = mybir.dt.bfloat16
AF = mybir.ActivationFunctionType


def _body(ctx: ExitStack, tc: tile.TileContext, x, qk, m2, g0, ident, sel, selt,
          out, sfx=""):
    nc = tc.nc

    def pool(name, **kw):
        return ctx.enter_context(tc.tile_pool(name=name + sfx, **kw))

    consts = pool("consts", bufs=1)
    qk0 = consts.tile([P, K], BF16, tag="qk0")
    qk1 = consts.tile([P, K], BF16, tag="qk1")
    m2t0 = consts.tile([P, C], F32, tag="m2t0")
    m2t1 = consts.tile([P, C], F32, tag="m2t1")
    g0_sb = consts.tile([P, 2], F32, tag="g0")
    id_sb = consts.tile([P, P], BF16, tag="ident")
    sel_sb = consts.tile([P, K], F32, tag="sel")    # sel[32j+k, k] = 1
    selt_sb = consts.tile([K, P], F32, tag="selt")  # its transpose

    xbig = pool("xbig", bufs=1)
    xres = {}
    n_big = HW // NB                 # 8
    for bt in range(n_big):
        xb0_t = xbig.tile([P, NB], BF16, tag=f"xb0_{bt}")
        xb1_t = xbig.tile([P, NB], BF16, tag=f"xb1_{bt}")
        xres[0, bt] = xb0_t
        xres[1, bt] = xb1_t

    # x tile 0 first (the pipeline head waits on it), early-needed consts next,
    # remaining x tiles split across the sync/scalar DGE queues, phase-B-only
    # consts last.
    nc.sync.dma_start(xres[0, 0][:], x[0:P, 0:NB])
    nc.sync.dma_start(xres[1, 0][:], x[P : 2 * P, 0:NB])
    nc.sync.dma_start(qk0[:], qk[0:P, :])
    nc.sync.dma_start(qk1[:], qk[P : 2 * P, :])
    nc.sync.dma_start(id_sb[:], ident[:, :])
    for bt in range(1, n_big):
        nc.sync.dma_start(xres[0, bt][:], x[0:P, bt * NB : (bt + 1) * NB])
        nc.sync.dma_start(xres[1, bt][:], x[P : 2 * P, bt * NB : (bt + 1) * NB])
    nc.sync.dma_start(m2t0[:], m2[0:P, :])
    nc.sync.dma_start(m2t1[:], m2[P : 2 * P, :])
    nc.sync.dma_start(g0_sb[:], g0[:, :])
    nc.sync.dma_start(sel_sb[:], sel[:, :])
    nc.sync.dma_start(selt_sb[:], selt[:, :])
    ps_l = pool("ps_l", bufs=2, space="PSUM")      # lT [128,84] f32
    ps_xt = pool("ps_xt", bufs=4, space="PSUM")    # xT half [128,2,256] bf16
    ps_y = pool("ps_y", bufs=1, space="PSUM")      # [y|s] accum [128,257] f32
    sb_xt = pool("sb_xt", bufs=4)
    sb_pt = pool("sb_pt", bufs=3)

    # ---------------- Phase A ------------------------------------------------
    # x stays resident in SBUF (16 tiles x 4 KiB/partition) -> phase C needs no
    # second HBM read. Per 512-px quad: 8 transposed-logits matmuls (x subtile
    # stationary, qk moving, 21 rows each) -> one exp [128,84] writing pT
    # straight to SBUF -> 8 x-transposes -> 4 y matmuls (col-group packed,
    # rhs gets a ones column so row 256 of PSUM accumulates s). The y matmuls
    # are emitted one quad late so the PE never stalls on the copy chain.
    # full-bank rows (512 f32 = 2048 B) so each col-group's partition offset is
    # zero-region aligned; cols past 257 are unused padding
    y_ps = ps_y.tile([P, 2 * C], F32, tag="y")
    nc.vector.memset(y_ps[:, :], 0.0)   # zero padding rows; y groups overwrite
    n_quad = NB // NQ                # 4
    n_sub = NQ // NS                 # 4
    n_t = HW // NQ                   # 32
    pend = None                      # deferred y-matmul args for quad t-1

    def emit_y(pt_sb, xt_sb, t):
        for j in range(n_sub):
            nc.tensor.matmul(
                y_ps[32 * j : 32 * j + K, 0 : C + 1],
                pt_sb[:, j * K : (j + 1) * K],
                xt_sb[:, j, :],
                start=(t == 0),
                stop=(t == n_t - 1),
                skip_group_check=True,
                tile_position=(0, 32 * j),
            )

    for bt in range(n_big):
        for g in range(n_quad):
            t = bt * n_quad + g
            xb0, xb1, base = xres[0, bt], xres[1, bt], g * NQ
            # transposed logits: lT[n, k] per 128-px subtile, both c-halves
            # accumulate. x subtile is the stationary operand.
            l_ps = ps_l.tile([P, n_sub * K], F32, tag="l")
            for j in range(n_sub):
                ss = slice(base + j * NS, base + (j + 1) * NS)
                osl = slice(j * K, (j + 1) * K)
                nc.tensor.matmul(l_ps[:, osl], xb0[:, ss], qk0[:],
                                 start=True, stop=False, skip_group_check=True)
                nc.tensor.matmul(l_ps[:, osl], xb1[:, ss], qk1[:],
                                 start=False, stop=True, skip_group_check=True)
            # exp -> pT directly in SBUF (bf16)
            pt_sb = sb_pt.tile([P, n_sub * K], BF16, tag="ptsb")
            nc.scalar.activation(pt_sb[:], l_ps[:], AF.Exp)

            # x transposes in two halves; copies all on DVE; ones column for s
            xt_sb = sb_xt.tile([P, n_sub, C + 1], BF16, tag="xtsb")
            nc.vector.memset(xt_sb[:, :, C : C + 1], 1.0)
            for h in range(2):
                xt_ps = ps_xt.tile([P, 2, C], BF16, tag="xt")
                for jj in range(2):
                    j = 2 * h + jj
                    ss = slice(base + j * NS, base + (j + 1) * NS)
                    nc.tensor.transpose(xt_ps[:, jj, 0:P], xb0[:, ss], id_sb[:, :])
                    nc.tensor.transpose(xt_ps[:, jj, P : 2 * P], xb1[:, ss], id_sb[:, :])
                nc.vector.tensor_copy(xt_sb[:, 2 * h : 2 * h + 2, 0:C], xt_ps[:])

            if pend is not None:
                emit_y(*pend)
            pend = (pt_sb, xt_sb, t)

    emit_y(*pend)

    # ---------------- Phase B: s -> r -> xa -> gate ---------------------------
    # y lives in 4 col-group blocks at partitions 32j..32j+20; each group holds
    # PARTIAL sums over its own subtile pixels, col 256 the partial s. Reduce
    # partial s across groups (sel.T @ s4), invert, replicate back to groups.
    s4_sb = consts.tile([P, 1], F32, tag="s4_sb")
    nc.vector.tensor_copy(s4_sb[:], y_ps[:, C : C + 1])
    yf_sb = consts.tile([P, C], F32, tag="yf_sb")
    nc.scalar.copy(yf_sb[:], y_ps[:, 0:C])
    sg_ps = ps_l.tile([K, 1], F32, tag="l")
    nc.tensor.matmul(sg_ps[:], sel_sb[:], s4_sb[:], start=True, stop=True)
    r_sb = consts.tile([K, 1], F32, tag="r_sb")
    nc.vector.reciprocal(r_sb[:], sg_ps[:])
    r4_ps = ps_l.tile([P, 1], F32, tag="l")
    nc.tensor.matmul(r4_ps[:], selt_sb[:], r_sb[:], start=True, stop=True)
    r4_sb = consts.tile([P, 1], F32, tag="r4_sb")
    nc.vector.tensor_copy(r4_sb[:], r4_ps[:])

    # xaT[m, h] = sum_p r4[p] yf[p, 128h+m]: two 1-col matmuls, no transposes
    xat_ps = ps_l.tile([P, 2], F32, tag="l")
    for h in range(2):
        nc.tensor.matmul(xat_ps[:, h : h + 1], yf_sb[:, h * P : (h + 1) * P],
                         r4_sb[:], start=True, stop=True, skip_group_check=True)
    xat_sb = consts.tile([P, 2], F32, tag="xat_sb")
    nc.vector.tensor_copy(xat_sb[:], xat_ps[:])

    gate_ps = ps_l.tile([P, 2], F32, tag="l")
    for cc in range(2):
        csl = slice(cc * P, (cc + 1) * P)
        nc.tensor.matmul(
            gate_ps[:, cc : cc + 1], m2t0[:, csl], xat_sb[:, 0:1],
            start=True, stop=False, skip_group_check=True,
        )
        nc.tensor.matmul(
            gate_ps[:, cc : cc + 1], m2t1[:, csl], xat_sb[:, 1:2],
            start=False, stop=True, skip_group_check=True,
        )
    gate_sb = consts.tile([P, 2], F32, tag="gate_sb")
    nc.vector.tensor_add(gate_sb[:], gate_ps[:], g0_sb[:])

    # ---------------- Phase C: out = x * gate (x already in SBUF) -------------
    for cc in range(2):
        csl = slice(cc * P, (cc + 1) * P)
        for nt in range(0, HW // NB):
            xc = xres[cc, nt]
            nsl = slice(nt * NB, (nt + 1) * NB)
            nc.vector.tensor_scalar_mul(xc[:], xc[:], gate_sb[:, cc : cc + 1])
            nc.sync.dma_start(out[csl, nsl], xc[:])


def build_nc(repeats=1, body=None):
    body = body or _body
    nc = bacc.Bacc(
        "TRN2",
        target_bir_lowering=False,
        debug=False,
        enable_asserts=False,
        num_devices=B,
    )
    x = nc.dram_tensor("x", [C, HW], BF16, kind="ExternalInput").ap()
    qk = nc.dram_tensor("qkT", [C, K], BF16, kind="ExternalInput").ap()
    m2 = nc.dram_tensor("m2t", [C, C], F32, kind="ExternalInput").ap()
    g0 = nc.dram_tensor("g0", [P, 2], F32, kind="ExternalInput").ap()
    ident = nc.dram_tensor("ident", [P, P], BF16, kind="ExternalInput").ap()
    sel = nc.dram_tensor("sel", [P, K], F32, kind="ExternalInput").ap()
    selt = nc.dram_tensor("selt", [K, P], F32, kind="ExternalInput").ap()
    out = nc.dram_tensor("out", [C, HW], BF16, kind="ExternalOutput").ap()

    with tile.TileContext(nc) as tc:
        for r in range(repeats):
            with ExitStack() as ctx:
                body(ctx, tc, x, qk, m2, g0, ident, sel, selt, out, sfx=f"_{r}")
    nc.compile()
    return nc


_NC = None


def _get_nc():
    global _NC
    if _NC is None:
        _NC = build_nc()
    return _NC


def make_in_maps(x, query_embed, Wk, bk, Wv, bv, Wo, bo):
    import ml_dtypes

    x = np.asarray(x, dtype=np.float32)
    qe = np.asarray(query_embed, dtype=np.float64)
    Wk64 = np.asarray(Wk, dtype=np.float64)
    Wv64 = np.asarray(Wv, dtype=np.float64)
    Wo64 = np.asarray(Wo, dtype=np.float64)
    bv64 = np.asarray(bv, dtype=np.float64)
    bo64 = np.asarray(bo, dtype=np.float64)

    qkT = ((qe @ Wk64) / np.sqrt(float(D))).T.astype(ml_dtypes.bfloat16).copy()
    m2t = (Wo64 @ Wv64).T.astype(np.float32).copy()
    g0 = (float(K) * (Wo64 @ bv64 + bo64)).astype(np.float32)
    g0c = np.ascontiguousarray(g0.reshape(2, P).T)
    ident = np.eye(P, dtype=ml_dtypes.bfloat16)
    sel = np.zeros((P, K), dtype=np.float32)
    for g in range(4):
        for k in range(K):
            sel[32 * g + k, k] = 1.0
    selt = np.ascontiguousarray(sel.T)

    return [
        {
            "x": np.ascontiguousarray(x[b].reshape(C, HW).astype(ml_dtypes.bfloat16)),
            "qkT": qkT,
            "m2t": m2t,
            "g0": g0c,
            "ident": ident,
            "sel": sel,
            "selt": selt,
        }
        for b in range(B)
    ]


def kernel(x, query_embed, Wk, bk, Wv, bv, Wo, bo, _trace=False, **kw):
    in_maps = make_in_maps(x, query_embed, Wk, bk, Wv, bv, Wo, bo)
    nc = _get_nc()
    res = run_bass_kernel_spmd(nc, in_maps, core_ids=list(range(B)), trace=_trace, **kw)
    out = np.stack(
        [np.asarray(res.results[b]["out"]).astype(np.float32).reshape(C, 128, 128)
         for b in range(B)]
    )
    if _trace:
        kernel.last_results = res
    return out
